# revision 1
# baseline (speedup 1.0000x reference)
"""Trainium2 Bass kernel: paged int8-KV-cache GQA decode attention, 8-core SPMD.

Contract: kernel(**inputs) takes the FULL unsharded numpy inputs (as produced by
the reference setup_inputs) and returns the FULL [32, 4096] float32 output.

Strategy (pure data parallel over sequences, per the sharding hint):
  - 32 decode sequences are sorted by context length and dealt across the
    8 cores (one per length-octile slot), so every core owns 4 sequences and
    runs an identical, statically-shaped program.
  - Host staging is permutation/layout only: the int8-valued int32 KV cache is
    gathered per block_tables into per-core packed buffers (K transposed to
    [kvh, d, tokens], V natural [kvh, tokens, d]) and uploaded as int32.
    The new decode token is quantized and scattered exactly as the reference's
    store_kvcache does, before the gather.
  - On device, SWDGE DMAs cast int32 -> bf16 inline during the HBM->SBUF load
    (no separate dequant pass); k_scale * softmax_scale and v_scale are folded
    in as per-token vectors after the QK matmul / after exp respectively.
  - Per (slot, kv_head, 128-token tile):
      scores^T [128t, 4h] = matmul(lhsT=K^T tile [128d,128t], rhs=q^T [128d,4h])
      s1 = scores^T * ksb  (DVE; ksb = k_scale*SCALE, zeroed beyond ctx)
      e  = exp(s1) in bf16 (ACT)
      em = e * mask01, ev = e * v_scale_vec (DVE)
      Z  = two-stage PE reduction of em over tokens (partition axis)
      out += matmul(lhsT=ev [128t,4h], rhs=V tile [128t,128d]) accumulated in
      PSUM; at slot end out_h = pv / Z.
  Softmax skips max-subtraction (scores are O(20) at most; fp32 exp is safe).
"""

import os
import sys
import math
from contextlib import ExitStack

import numpy as np

sys.path.insert(0, "/opt/trn_rl_repo")

import ml_dtypes  # noqa: E402

import concourse.bass as bass  # noqa: E402
import concourse.mybir as mybir  # noqa: E402
import concourse.tile as tile  # noqa: E402
from concourse import bacc  # noqa: E402
from concourse.bass_utils import run_bass_kernel_spmd  # noqa: E402

BF16 = ml_dtypes.bfloat16

B = 32
NUM_HEADS = 32
KVH = 8
D = 128
REP = NUM_HEADS // KVH  # 4
BLOCK_SIZE = 256
T = 4096
P = 128
SCALE = 1.0 / float(np.sqrt(D))
NCORES = 8
SLOTS = 4


# ---------------------------------------------------------------------------
# host-side planning + packing
# ---------------------------------------------------------------------------

def _plan(context_lens):
    """Assign sequences to (core, slot); slot tile counts = octile maxima.

    Each slot's K/V loads are split into up to 3 pieces; pieces past a
    sequence's length are skipped per-core via predicated DMAs.  Returns
    (assign, ns, pieces) where pieces[s] is the list of piece boundaries.
    """
    order = np.argsort(-context_lens, kind="stable")  # descending
    ns = []
    assign = np.zeros((NCORES, SLOTS), dtype=np.int64)
    for s in range(SLOTS):
        octile = order[8 * s: 8 * s + 8]
        ns.append(int(math.ceil(int(context_lens[octile[0]]) / P)))
        # alternate direction per slot to roughly balance true work
        ranks = octile if s % 2 == 0 else octile[::-1]
        for c in range(NCORES):
            assign[c, s] = ranks[c]

    # NOTE: runtime-predicated (cond=) piece skipping was tried to trim the
    # ~12% slot padding, but SWDGE cond-DMAs produced wrong data on HW even
    # with always-true flags, so loads are unconditional.
    pieces = [[0, n] for n in ns]
    return assign, ns, pieces


def _quantize(x):
    absmax = np.abs(x).max(axis=-1)
    scale = np.where(absmax > 0.0, absmax / 127.0, 1.0).astype(np.float32)
    xq = np.clip(np.round(x / scale[..., None]), -127.0, 127.0).astype(np.int32)
    return xq, scale


def _pack_inputs(inputs, assign, ns, pieces):
    q = inputs["q"].reshape(B, NUM_HEADS, D).astype(np.float32)
    k = inputs["k"].reshape(B, KVH, D).astype(np.float32)
    v = inputs["v"].reshape(B, KVH, D).astype(np.float32)
    kc = np.ascontiguousarray(inputs["k_cache_q"].reshape(-1, KVH, D))
    vc = np.ascontiguousarray(inputs["v_cache_q"].reshape(-1, KVH, D))
    ks = np.ascontiguousarray(inputs["k_scale"].reshape(-1, KVH)).astype(np.float32)
    vs = np.ascontiguousarray(inputs["v_scale"].reshape(-1, KVH)).astype(np.float32)
    bt = inputs["block_tables"]
    ctx = inputs["context_lens"]
    sm = inputs["slot_mapping"]

    # store_kvcache_int8: quantize the new token and scatter into the cache
    kq, ksn = _quantize(k)
    vq, vsn = _quantize(v)
    kc = kc.copy(); vc = vc.copy(); ks = ks.copy(); vs = vs.copy()
    kc[sm] = kq; vc[sm] = vq; ks[sm] = ksn; vs[sm] = vsn

    NTT = sum(ns)           # token tiles per core
    NT = NTT * P            # tokens per core
    offs = np.concatenate([[0], np.cumsum(ns)])

    in_maps = []
    for c in range(NCORES):
        kt_c = np.zeros((KVH, D, NT), dtype=np.int32)
        # V pre-tiled [kvh, partition, tile, d] so each partition's slot data
        # is one contiguous run for the DMA (avoids 256B-packet spray)
        vp_c = np.zeros((KVH, P, NTT, D), dtype=np.int32)
        ksb_c = np.zeros((P, NTT * KVH), dtype=np.float32)
        vsb_c = np.zeros((P, NTT * KVH), dtype=BF16)
        m01_c = np.zeros((P, NTT * KVH), dtype=BF16)
        qt_c = np.zeros((P, SLOTS * 32), dtype=np.float32)
        pf_c = np.zeros((1, 8), dtype=np.int32)
        for s in range(SLOTS):
            b = int(assign[c, s])
            n = ns[s]
            nt = n * P
            o = int(offs[s])
            act = int(math.ceil(int(inputs["context_lens"][b]) / P))
            for pi, st in enumerate(pieces[s][1:-1]):
                pf_c[0, (s - 1) * 2 + pi] = 1 if act > st else 0
            flat = (bt[b][:, None] * BLOCK_SIZE
                    + np.arange(BLOCK_SIZE, dtype=np.int64)[None, :]).reshape(-1)[:nt]
            kg = kc[flat]                      # [nt, KVH, D] int32
            vg = vc[flat]
            kt_c[:, :, o * P: o * P + nt] = kg.transpose(1, 2, 0)
            # [nt, KVH, D] -> [n, P, KVH, D] -> [KVH, P, n, D]
            vp_c[:, :, o: o + n, :] = vg.reshape(n, P, KVH, D).transpose(2, 1, 0, 3)
            valid = (np.arange(nt) < int(ctx[b]))
            ksg = (ks[flat] * SCALE) * valid[:, None]        # [nt, KVH]
            vsg = vs[flat] * valid[:, None]
            # [nt, KVH] -> [P, n*KVH]  (head-broadcast happens on device)
            def sprd(a, dt):
                return a.reshape(n, P, KVH).transpose(1, 0, 2).reshape(
                    P, n * KVH).astype(dt)
            ksb_c[:, o * KVH: (o + n) * KVH] = sprd(ksg, np.float32)
            vsb_c[:, o * KVH: (o + n) * KVH] = sprd(vsg, BF16)
            m01_c[:, o * KVH: (o + n) * KVH] = sprd(
                valid[:, None] * np.ones((1, KVH), np.float32), BF16)
            qt_c[:, s * 32: (s + 1) * 32] = q[b].transpose(1, 0)  # [D, 32]
        sel = np.tile(np.eye(4, dtype=np.float32), (32, 1))       # [128, 4]
        in_maps.append(dict(kt=kt_c, vp=vp_c, ksb=ksb_c, vsb=vsb_c,
                            m01=m01_c, qt=qt_c, sel=sel, pf=pf_c))
    return in_maps


# ---------------------------------------------------------------------------
# device program
# ---------------------------------------------------------------------------

def _kvh_body(nc, s, n, j, ktc, vtc, qt, sel, ones, ksb_s, vsb_s, m01_s,
              ps_qk, ps_pt, work, zts, pvts):
    """QK -> scale -> exp -> mask -> Z -> PV for one (slot, kv head)."""
    f32 = mybir.dt.float32
    bf16 = mybir.dt.bfloat16
    EXP = mybir.ActivationFunctionType.Exp

    qk = ps_qk.tile([P, n, 4], f32, tag="qk")
    qcol = s * 32 + 4 * j
    for i in range(n):
        nc.tensor.matmul(
            qk[:, i, :],
            lhsT=ktc[:, i, :],
            rhs=qt[:, qcol: qcol + 4],
            start=True, stop=True, skip_group_check=True)

    s1 = work.tile([P, n, 4], f32, tag="s1")
    nc.vector.tensor_mul(
        s1, qk, ksb_s[:, :, j: j + 1].to_broadcast([P, n, 4]))
    e = work.tile([P, n, 4], bf16, tag="e")
    nc.scalar.activation(e, s1, EXP)
    em = work.tile([P, n, 4], bf16, tag="em")
    nc.vector.tensor_mul(
        em, e, m01_s[:, :, j: j + 1].to_broadcast([P, n, 4]))
    ev = work.tile([P, n, 4], bf16, tag="ev")
    nc.vector.tensor_mul(
        ev, e, vsb_s[:, :, j: j + 1].to_broadcast([P, n, 4]))

    # Z: per-(tile, head) partial sums, then fold tiles via selector matmul
    pt = ps_pt.tile([P, 1], f32, tag="pt")
    nc.tensor.matmul(pt[0: n * 4, :], lhsT=em, rhs=ones,
                     start=True, stop=True)
    pts = work.tile([P, 1], f32, tag="pts")
    nc.vector.tensor_copy(pts[0: n * 4, :], pt[0: n * 4, :])
    zt = zts[j // 4]
    bp = 32 * (j % 4)
    nc.tensor.matmul(zt[bp: bp + 4, :], lhsT=sel[0: n * 4, :],
                     rhs=pts[0: n * 4, :], start=True, stop=True,
                     tile_position=(0, bp))

    # PV accumulate over token tiles
    pvt = pvts[j // 4]
    for i in range(n):
        nc.tensor.matmul(
            pvt[bp: bp + 4, :],
            lhsT=ev[:, i, :],
            rhs=vtc[:, i, :],
            start=(i == 0), stop=(i == n - 1),
            tile_position=(0, bp))


def _build_program(ns, pieces, no_cond=False, no_memset=False):
    NTT = sum(ns)
    NT = NTT * P
    offs = [0]
    for n in ns:
        offs.append(offs[-1] + n)
    f32 = mybir.dt.float32
    bf16 = mybir.dt.bfloat16
    i32 = mybir.dt.int32
    EXP = mybir.ActivationFunctionType.Exp

    nc = bacc.Bacc("TRN2", target_bir_lowering=False, debug=False,
                   num_devices=NCORES)

    kt_d = nc.dram_tensor("kt", [KVH, D, NT], i32, kind="ExternalInput").ap()
    vp_d = nc.dram_tensor("vp", [KVH, P, NTT, D], i32, kind="ExternalInput").ap()
    ksb_d = nc.dram_tensor("ksb", [P, NTT * KVH], f32, kind="ExternalInput").ap()
    vsb_d = nc.dram_tensor("vsb", [P, NTT * KVH], bf16, kind="ExternalInput").ap()
    m01_d = nc.dram_tensor("m01", [P, NTT * KVH], bf16, kind="ExternalInput").ap()
    qt_d = nc.dram_tensor("qt", [P, SLOTS * 32], f32, kind="ExternalInput").ap()
    sel_d = nc.dram_tensor("sel", [P, 4], f32, kind="ExternalInput").ap()
    pf_d = nc.dram_tensor("pf", [1, 8], mybir.dt.int32,
                          kind="ExternalInput").ap()
    out_d = nc.dram_tensor("out", [SLOTS, 2, P, D], f32,
                           kind="ExternalOutput").ap()

    with tile.TileContext(nc) as tc, ExitStack() as ctx:
        const = ctx.enter_context(tc.tile_pool(name="const", bufs=1))
        kt_pool = ctx.enter_context(tc.tile_pool(name="ktp", bufs=2))
        v_pool = ctx.enter_context(tc.tile_pool(name="vpp", bufs=2))
        sc_pool = ctx.enter_context(tc.tile_pool(name="scp", bufs=2))
        work = ctx.enter_context(tc.tile_pool(name="wrk", bufs=3))
        ps_qk = ctx.enter_context(tc.tile_pool(name="psqk", bufs=2, space="PSUM"))
        ps_pt = ctx.enter_context(tc.tile_pool(name="pspt", bufs=2, space="PSUM"))
        ps_z = ctx.enter_context(tc.tile_pool(name="psz", bufs=1, space="PSUM"))
        ps_pv = ctx.enter_context(tc.tile_pool(name="pspv", bufs=1, space="PSUM"))

        qt_f = const.tile([P, SLOTS * 32], f32)
        nc.sync.dma_start(qt_f, qt_d)
        qt = const.tile([P, SLOTS * 32], bf16)
        nc.vector.tensor_copy(qt, qt_f)
        sel = const.tile([P, 4], f32)
        nc.sync.dma_start(sel, sel_d)
        ones = const.tile([P, 1], bf16)
        nc.vector.memset(ones, 1.0)
        conds = {}
        if any(len(p) > 2 for p in pieces):
            pf_sb = const.tile([1, 8], mybir.dt.int32)
            nc.sync.dma_start(pf_sb, pf_d)
            for s in range(1, SLOTS):
                for pi in range(len(pieces[s]) - 2):
                    reg = nc.alloc_registers(f"pf_{s}_{pi}",
                                             engines=[mybir.EngineType.Pool])
                    nc.regs_load(reg, pf_sb[0:1, (s - 1) * 2 + pi:
                                            (s - 1) * 2 + pi + 1])
                    conds[(s, pi)] = nc.snap(reg, donate=True,
                                             min_val=0, max_val=1)

        for s in range(SLOTS):
            n = ns[s]
            o = offs[s]
            ksb_s = sc_pool.tile([P, n, KVH], f32, tag="ksb")
            nc.sync.dma_start(ksb_s, ksb_d[:, o * KVH: (o + n) * KVH])
            vsb_s = sc_pool.tile([P, n, KVH], bf16, tag="vsb")
            nc.sync.dma_start(vsb_s, vsb_d[:, o * KVH: (o + n) * KVH])
            m01_s = sc_pool.tile([P, n, KVH], bf16, tag="m01")
            nc.sync.dma_start(m01_s, m01_d[:, o * KVH: (o + n) * KVH])

            pv0 = ps_pv.tile([P, D], f32, tag="pv0")
            pv1 = ps_pv.tile([P, D], f32, tag="pv1")
            z0 = ps_z.tile([P, 1], f32, tag="z0")
            z1 = ps_z.tile([P, 1], f32, tag="z1")
            nc.vector.memset(pv0, 0.0)
            nc.vector.memset(pv1, 0.0)
            nc.vector.memset(z0, 1.0)
            nc.vector.memset(z1, 1.0)

            for jq in range(KVH // 4):
                ktc = kt_pool.tile([P, 4, n, P], bf16, tag="kt")
                vtc = v_pool.tile([P, 4, n, D], bf16, tag="vt")
                nc.gpsimd.dma_start(
                    ktc,
                    kt_d[4 * jq: 4 * jq + 4, :,
                         o * P: (o + n) * P].rearrange("j d t -> d j t"))
                nc.gpsimd.dma_start(
                    vtc,
                    vp_d[4 * jq: 4 * jq + 4, :, o: o + n,
                         :].rearrange("j p i d -> p j i d"))
                for j2 in range(4):
                    _kvh_body(nc, s, n, 4 * jq + j2,
                              ktc[:, j2], vtc[:, j2],
                              qt, sel, ones, ksb_s, vsb_s, m01_s,
                              ps_qk, ps_pt, work,
                              (z0, z1), (pv0, pv1))

            rz0 = work.tile([P, 1], f32, tag="rz0")
            nc.vector.reciprocal(rz0, z0)
            rz1 = work.tile([P, 1], f32, tag="rz1")
            nc.vector.reciprocal(rz1, z1)
            o0 = work.tile([P, D], f32, tag="o0")
            nc.vector.tensor_scalar_mul(o0, pv0, rz0)
            o1 = work.tile([P, D], f32, tag="o1")
            nc.vector.tensor_scalar_mul(o1, pv1, rz1)
            nc.sync.dma_start(out_d[s, 0], o0)
            nc.sync.dma_start(out_d[s, 1], o1)

    nc.compile()
    return nc


_PROGRAM_CACHE = {}


def _get_program(ns, pieces):
    key = (tuple(ns), tuple(tuple(p) for p in pieces))
    if key not in _PROGRAM_CACHE:
        _PROGRAM_CACHE[key] = _build_program(ns, pieces)
    return _PROGRAM_CACHE[key]


# ---------------------------------------------------------------------------
# entry point
# ---------------------------------------------------------------------------

def kernel(q, k, v, k_cache_q, v_cache_q, k_scale, v_scale,
           block_tables, context_lens, slot_mapping, _trace=False):
    inputs = dict(q=np.asarray(q), k=np.asarray(k), v=np.asarray(v),
                  k_cache_q=np.asarray(k_cache_q),
                  v_cache_q=np.asarray(v_cache_q),
                  k_scale=np.asarray(k_scale), v_scale=np.asarray(v_scale),
                  block_tables=np.asarray(block_tables),
                  context_lens=np.asarray(context_lens),
                  slot_mapping=np.asarray(slot_mapping))
    assign, ns, pieces = _plan(inputs["context_lens"])
    in_maps = _pack_inputs(inputs, assign, ns, pieces)
    nc = _get_program(ns, pieces)
    res = run_bass_kernel_spmd(nc, in_maps, core_ids=list(range(NCORES)),
                               trace=_trace)

    out = np.zeros((B, NUM_HEADS, D), dtype=np.float32)
    for c in range(NCORES):
        oc = res.results[c]["out"]  # [SLOTS, 2, P, D]
        for s in range(SLOTS):
            b = int(assign[c, s])
            for j in range(KVH):
                bp = 32 * (j % 4)
                out[b, 4 * j: 4 * j + 4] = oc[s, j // 4, bp: bp + 4, :]
    out = out.reshape(B, NUM_HEADS * D)
    if _trace:
        return out, res
    return out



# revision 2
# speedup vs baseline: 1.9558x; 1.9558x over previous
"""Trainium2 Bass kernel: paged int8-KV-cache GQA decode attention, 8-core SPMD.

Contract: kernel(**inputs) takes the FULL unsharded numpy inputs (as produced by
the reference setup_inputs) and returns the FULL [32, 4096] float32 output.

Strategy (pure data parallel over sequences, per the sharding hint):
  - 32 decode sequences are sorted by context length and dealt across the
    8 cores (one per length-octile slot), so every core owns 4 sequences and
    runs an identical, statically-shaped program.
  - Host staging is permutation/layout only: K/V int8 cache values are
    gathered per block_tables into per-core packed int8 buffers (1 byte per
    element in HBM; the old kernel moved them as int32 and was 4x the DMA).
    The new decode token is quantized and scattered exactly as the
    reference's store_kvcache does, before the gather.
  - On device, SWDGE DMAs cast int8 -> bf16 inline (exact: values are
    +-127 integers).  k_scale*softmax_scale (f32) multiplies the QK psum
    per token; v_scale is folded into the probabilities.
  - Per (slot, kvh-group of 4):
      QK:  for each 128-token tile i and kvh j2: scores^T [128t, 4h] =
           matmul(lhsT=K^T tile [128d,128t], rhs=q^T [128d,4h]) into a
           per-chunk psum bank [128, n, 16].
      one DVE mul (qk *= ksb) + one ACT exp -> e bf16, one DVE mul -> ev =
           e * v_scale (all batched per chunk, not per tile).
      PV+Z fused: matmul(lhsT=[e|ev] [128t, 8], rhs=[V|mask] [128t, 129])
           accumulated over tiles; kvh j2 lands on PE column-group j2
           (tile_position) so the 4 streams overlap.  Rows 32*j2+0..3 hold
           Z (col 128); rows 32*j2+4..7 hold PV (cols 0..127).
  - The final divide pv/Z happens on the host during unpacking.
  Softmax skips max-subtraction (scores are O(20) at most; fp32 exp is safe).
"""

import math
import sys
from contextlib import ExitStack

import numpy as np

sys.path.insert(0, "/opt/trn_rl_repo")

import ml_dtypes  # noqa: E402

import concourse.bass as bass  # noqa: E402
import concourse.mybir as mybir  # noqa: E402
import concourse.tile as tile  # noqa: E402
from concourse import bacc  # noqa: E402
from concourse.bass_utils import run_bass_kernel_spmd  # noqa: E402

BF16 = ml_dtypes.bfloat16

B = 32
NUM_HEADS = 32
KVH = 8
D = 128
REP = NUM_HEADS // KVH  # 4
BLOCK_SIZE = 256
T = 4096
P = 128
DV = D + 1  # V columns + mask column
SCALE = 1.0 / float(np.sqrt(D))
NCORES = 8
SLOTS = 4


# ---------------------------------------------------------------------------
# host-side planning + packing
# ---------------------------------------------------------------------------

def _plan(context_lens):
    """Assign sequences to (core, slot); slot tile counts = octile maxima."""
    order = np.argsort(-context_lens, kind="stable")  # descending
    ns = []
    assign = np.zeros((NCORES, SLOTS), dtype=np.int64)
    for s in range(SLOTS):
        octile = order[8 * s: 8 * s + 8]
        ns.append(int(math.ceil(int(context_lens[octile[0]]) / P)))
        ranks = octile if s % 2 == 0 else octile[::-1]
        for c in range(NCORES):
            assign[c, s] = ranks[c]
    return assign, ns


def _quantize(x):
    absmax = np.abs(x).max(axis=-1)
    scale = np.where(absmax > 0.0, absmax / 127.0, 1.0).astype(np.float32)
    xq = np.clip(np.round(x / scale[..., None]), -127.0, 127.0).astype(np.int8)
    return xq, scale


def _pack_inputs(inputs, assign, ns):
    q = inputs["q"].reshape(B, NUM_HEADS, D).astype(np.float32)
    k = inputs["k"].reshape(B, KVH, D).astype(np.float32)
    v = inputs["v"].reshape(B, KVH, D).astype(np.float32)
    kc = np.ascontiguousarray(
        inputs["k_cache_q"].reshape(-1, KVH, D).astype(np.int8))
    vc = np.ascontiguousarray(
        inputs["v_cache_q"].reshape(-1, KVH, D).astype(np.int8))
    ks = np.ascontiguousarray(inputs["k_scale"].reshape(-1, KVH)).astype(np.float32)
    vs = np.ascontiguousarray(inputs["v_scale"].reshape(-1, KVH)).astype(np.float32)
    bt = inputs["block_tables"]
    ctx = inputs["context_lens"]
    sm = inputs["slot_mapping"]

    # store_kvcache_int8: quantize the new token and scatter into the cache
    kq, ksn = _quantize(k)
    vq, vsn = _quantize(v)
    kc = kc.copy(); vc = vc.copy(); ks = ks.copy(); vs = vs.copy()
    kc[sm] = kq; vc[sm] = vq; ks[sm] = ksn; vs[sm] = vsn

    NTT = sum(ns)           # token tiles per core
    NT = NTT * P            # tokens per core
    offs = np.concatenate([[0], np.cumsum(ns)])

    in_maps = []
    for c in range(NCORES):
        kt_c = np.zeros((KVH, D, NT), dtype=np.int8)
        # V pre-tiled [kvh, partition(token%128), tile, DV]; col D is mask
        vp_c = np.zeros((KVH, P, NTT, DV), dtype=np.int8)
        ksb_c = np.zeros((P, 2, NTT, 4), dtype=np.float32)
        vsb_c = np.zeros((P, 2, NTT, 4), dtype=BF16)
        qt_c = np.zeros((P, SLOTS * 32), dtype=BF16)
        for s in range(SLOTS):
            b = int(assign[c, s])
            n = ns[s]
            nt = n * P
            o = int(offs[s])
            cl = int(ctx[b])
            flat = (bt[b][:, None] * BLOCK_SIZE
                    + np.arange(BLOCK_SIZE, dtype=np.int64)[None, :]).reshape(-1)[:nt]
            valid = (np.arange(nt) < cl)
            kg = kc[flat] * valid[:, None, None]   # [nt, KVH, D] int8, 0 pad
            vg = vc[flat] * valid[:, None, None]
            kt_c[:, :, o * P: o * P + nt] = kg.transpose(1, 2, 0)
            vp_c[:, :, o: o + n, :D] = vg.reshape(n, P, KVH, D).transpose(2, 1, 0, 3)
            vp_c[:, :, o: o + n, D] = valid.reshape(n, P).transpose(1, 0)[None]
            ksg = (ks[flat] * SCALE) * valid[:, None]        # [nt, KVH]
            vsg = vs[flat] * valid[:, None]
            # [nt, KVH] -> [P(tok%128), 2, n, 4]
            ksb_c[:, :, o: o + n, :] = (
                ksg.reshape(n, P, 2, 4).transpose(1, 2, 0, 3))
            vsb_c[:, :, o: o + n, :] = (
                vsg.reshape(n, P, 2, 4).transpose(1, 2, 0, 3).astype(BF16))
            qt_c[:, s * 32: (s + 1) * 32] = q[b].transpose(1, 0).astype(BF16)
        in_maps.append(dict(kt=kt_c, vp=vp_c, ksb=ksb_c, vsb=vsb_c, qt=qt_c))
    return in_maps


# ---------------------------------------------------------------------------
# device program
# ---------------------------------------------------------------------------

def _build_program(ns):
    NTT = sum(ns)
    NT = NTT * P
    offs = [0]
    for n in ns:
        offs.append(offs[-1] + n)
    f32 = mybir.dt.float32
    bf16 = mybir.dt.bfloat16
    i8 = mybir.dt.int8
    EXP = mybir.ActivationFunctionType.Exp

    nc = bacc.Bacc("TRN2", target_bir_lowering=False, debug=False,
                   num_devices=NCORES)

    kt_d = nc.dram_tensor("kt", [KVH, D, NT], i8, kind="ExternalInput").ap()
    vp_d = nc.dram_tensor("vp", [KVH, P, NTT, DV], i8, kind="ExternalInput").ap()
    ksb_d = nc.dram_tensor("ksb", [P, 2, NTT, 4], f32, kind="ExternalInput").ap()
    vsb_d = nc.dram_tensor("vsb", [P, 2, NTT, 4], bf16, kind="ExternalInput").ap()
    qt_d = nc.dram_tensor("qt", [P, SLOTS * 32], bf16, kind="ExternalInput").ap()
    out_d = nc.dram_tensor("out", [SLOTS, 2, P, DV], f32,
                           kind="ExternalOutput").ap()

    with tile.TileContext(nc) as tc, ExitStack() as ctx:
        const = ctx.enter_context(tc.tile_pool(name="const", bufs=1))
        kt_pool = ctx.enter_context(tc.tile_pool(name="ktp", bufs=2))
        v_pool = ctx.enter_context(tc.tile_pool(name="vpp", bufs=2))
        sc_pool = ctx.enter_context(tc.tile_pool(name="scp", bufs=2))
        work = ctx.enter_context(tc.tile_pool(name="wrk", bufs=3))
        o_pool = ctx.enter_context(tc.tile_pool(name="osb", bufs=2))
        ps_qk = ctx.enter_context(tc.tile_pool(name="psqk", bufs=2, space="PSUM"))
        ps_pv = ctx.enter_context(tc.tile_pool(name="pspv", bufs=2, space="PSUM"))

        qt = const.tile([P, SLOTS * 32], bf16)
        nc.sync.dma_start(qt, qt_d)

        for s in range(SLOTS):
            n = ns[s]
            o = offs[s]
            for g in range(2):
                kc = kt_pool.tile([P, 4, n, P], bf16, tag="kt")
                nc.gpsimd.dma_start(
                    kc,
                    kt_d[4 * g: 4 * g + 4, :,
                         o * P: (o + n) * P].rearrange("j d t -> d j t"))
                vc = v_pool.tile([P, 4, n, DV], bf16, tag="vt")
                nc.gpsimd.dma_start(
                    vc,
                    vp_d[4 * g: 4 * g + 4, :, o: o + n,
                         :].rearrange("j p i c -> p j i c"))
                ksb_t = sc_pool.tile([P, n, 4], f32, tag="ksb")
                nc.sync.dma_start(ksb_t, ksb_d[:, g, o: o + n, :])
                vsb_t = sc_pool.tile([P, n, 4], bf16, tag="vsb")
                nc.sync.dma_start(vsb_t, vsb_d[:, g, o: o + n, :])

                qk = ps_qk.tile([P, n, 4, 4], f32, tag="qk")
                for i in range(n):
                    for j2 in range(4):
                        qcol = s * 32 + (4 * g + j2) * 4
                        nc.tensor.matmul(
                            qk[:, i, j2, :],
                            lhsT=kc[:, j2, i, :],
                            rhs=qt[:, qcol: qcol + 4],
                            start=True, stop=True, skip_group_check=True)

                # s1 = qk * ksb (broadcast over the 4 head columns), in psum
                nc.vector.tensor_mul(
                    qk, qk,
                    ksb_t.unsqueeze(3).to_broadcast([P, n, 4, 4]))
                ew = work.tile([P, n, 4, 8], bf16, tag="ew")
                nc.scalar.activation(ew[:, :, :, 0:4], qk, EXP)
                nc.vector.tensor_mul(
                    ew[:, :, :, 4:8], ew[:, :, :, 0:4],
                    vsb_t.unsqueeze(3).to_broadcast([P, n, 4, 4]))

                pv = ps_pv.tile([P, DV], f32, tag="pv")
                nc.vector.memset(pv, 0.0)
                for i in range(n):
                    for j2 in range(4):
                        nc.tensor.matmul(
                            pv[32 * j2: 32 * j2 + 8, :],
                            lhsT=ew[:, i, j2, :],
                            rhs=vc[:, j2, i, :],
                            start=(i == 0), stop=(i == n - 1),
                            tile_position=(0, 32 * j2),
                            skip_group_check=True)

                osb = o_pool.tile([P, DV], f32, tag="osb")
                nc.vector.tensor_copy(osb, pv)
                nc.sync.dma_start(out_d[s, g], osb)

    nc.compile()
    return nc


_PROGRAM_CACHE = {}


def _get_program(ns):
    key = tuple(ns)
    if key not in _PROGRAM_CACHE:
        _PROGRAM_CACHE[key] = _build_program(ns)
    return _PROGRAM_CACHE[key]


# ---------------------------------------------------------------------------
# entry point
# ---------------------------------------------------------------------------

def kernel(q, k, v, k_cache_q, v_cache_q, k_scale, v_scale,
           block_tables, context_lens, slot_mapping, _trace=False):
    inputs = dict(q=np.asarray(q), k=np.asarray(k), v=np.asarray(v),
                  k_cache_q=np.asarray(k_cache_q),
                  v_cache_q=np.asarray(v_cache_q),
                  k_scale=np.asarray(k_scale), v_scale=np.asarray(v_scale),
                  block_tables=np.asarray(block_tables),
                  context_lens=np.asarray(context_lens),
                  slot_mapping=np.asarray(slot_mapping))
    assign, ns = _plan(inputs["context_lens"])
    in_maps = _pack_inputs(inputs, assign, ns)
    nc = _get_program(ns)
    res = run_bass_kernel_spmd(nc, in_maps, core_ids=list(range(NCORES)),
                               trace=_trace)

    out = np.zeros((B, NUM_HEADS, D), dtype=np.float32)
    for c in range(NCORES):
        oc = res.results[c]["out"]  # [SLOTS, 2, P, DV] f32
        for s in range(SLOTS):
            b = int(assign[c, s])
            for g in range(2):
                for j2 in range(4):
                    j = 4 * g + j2
                    z = oc[s, g, 32 * j2: 32 * j2 + 4, D]          # [4]
                    pvv = oc[s, g, 32 * j2 + 4: 32 * j2 + 8, :D]   # [4, D]
                    out[b, 4 * j: 4 * j + 4] = pvv / z[:, None]
    out = out.reshape(B, NUM_HEADS * D)
    if _trace:
        return out, res
    return out


# revision 3
# speedup vs baseline: 2.0356x; 1.0408x over previous
"""Trainium2 Bass kernel: paged int8-KV-cache GQA decode attention, 8-core SPMD.

Contract: kernel(**inputs) takes the FULL unsharded numpy inputs (as produced by
the reference setup_inputs) and returns the FULL [32, 4096] float32 output.

Strategy (pure data parallel over sequences, per the sharding hint):
  - 32 decode sequences are sorted by context length and dealt across the
    8 cores (one per length-octile slot), so every core owns 4 sequences and
    runs an identical, statically-shaped program.
  - K/V int8 cache values are gathered per block_tables into per-core packed
    int8 buffers (1 byte per element in HBM), laid out block-major so every
    HBM->SBUF DMA is one contiguous run per partition.  SWDGE DMAs cast
    int8 -> bf16 inline (exact: values are +-127 integers).
  - Work is chopped into <=BT-token-tile blocks per (slot, kvh-group); a tiny
    lead block primes the pipeline so the PE starts ~2us in.
  - Per block: QK matmuls (K^T tile as stationary operand) -> one DVE mul by
    k_scale*softmax_scale -> one ACT exp -> one DVE mul by v_scale; then the
    PV+Z fused matmul lhsT=[e|ev], rhs=[V|mask] accumulated into a per-
    (slot,group) PSUM bank, kvh j2 on PE column-group j2 (tile_position).
    Rows 32*j2+0..3 hold Z (col 128); rows 32*j2+4..7 hold PV (cols 0..127).
  - The final divide pv/Z happens on the host during unpacking.
  Softmax skips max-subtraction (scores are O(20) at most; fp32 exp is safe).
"""

import math
import sys
from contextlib import ExitStack

import numpy as np

sys.path.insert(0, "/opt/trn_rl_repo")

import ml_dtypes  # noqa: E402

import concourse.bass as bass  # noqa: E402
import concourse.mybir as mybir  # noqa: E402
import concourse.tile as tile  # noqa: E402
from concourse import bacc  # noqa: E402
from concourse.bass_utils import run_bass_kernel_spmd  # noqa: E402

BF16 = ml_dtypes.bfloat16

B = 32
NUM_HEADS = 32
KVH = 8
D = 128
REP = NUM_HEADS // KVH  # 4
BLOCK_SIZE = 256
T = 4096
P = 128
DV = D + 1  # V columns + mask column
SCALE = 1.0 / float(np.sqrt(D))
NCORES = 8
SLOTS = 4
BT = 12     # token tiles per pipeline block


# ---------------------------------------------------------------------------
# host-side planning + packing
# ---------------------------------------------------------------------------

def _plan(context_lens):
    """Assign sequences to (core, slot); slot tile counts = octile maxima."""
    order = np.argsort(-context_lens, kind="stable")  # descending
    ns = []
    assign = np.zeros((NCORES, SLOTS), dtype=np.int64)
    for s in range(SLOTS):
        octile = order[8 * s: 8 * s + 8]
        ns.append(int(math.ceil(int(context_lens[octile[0]]) / P)))
        ranks = octile if s % 2 == 0 else octile[::-1]
        for c in range(NCORES):
            assign[c, s] = ranks[c]
    return assign, ns


def _blocks(ns):
    """[(s, g, tile_off_in_slot, bt)], with a small lead block to prime."""
    out = []
    for s in range(SLOTS):
        n = ns[s]
        for g in range(2):
            bo = 0
            if s == 0 and g == 0 and n > 2:
                out.append((s, g, 0, 2))
                bo = 2
            while bo < n:
                bt = min(BT, n - bo)
                out.append((s, g, bo, bt))
                bo += bt
    return out


def _quantize(x):
    absmax = np.abs(x).max(axis=-1)
    scale = np.where(absmax > 0.0, absmax / 127.0, 1.0).astype(np.float32)
    xq = np.clip(np.round(x / scale[..., None]), -127.0, 127.0).astype(np.int8)
    return xq, scale


def _pack_inputs(inputs, assign, ns, blocks):
    q = inputs["q"].reshape(B, NUM_HEADS, D).astype(np.float32)
    k = inputs["k"].reshape(B, KVH, D).astype(np.float32)
    v = inputs["v"].reshape(B, KVH, D).astype(np.float32)
    kc = np.ascontiguousarray(
        inputs["k_cache_q"].reshape(-1, KVH, D).astype(np.int8))
    vc = np.ascontiguousarray(
        inputs["v_cache_q"].reshape(-1, KVH, D).astype(np.int8))
    ks = np.ascontiguousarray(inputs["k_scale"].reshape(-1, KVH)).astype(np.float32)
    vs = np.ascontiguousarray(inputs["v_scale"].reshape(-1, KVH)).astype(np.float32)
    bt_tab = inputs["block_tables"]
    ctx = inputs["context_lens"]
    sm = inputs["slot_mapping"]

    # store_kvcache_int8: quantize the new token and scatter into the cache
    kq, ksn = _quantize(k)
    vq, vsn = _quantize(v)
    kc = kc.copy(); vc = vc.copy(); ks = ks.copy(); vs = vs.copy()
    kc[sm] = kq; vc[sm] = vq; ks[sm] = ksn; vs[sm] = vsn

    NTT = sum(ns)
    offs = np.concatenate([[0], np.cumsum(ns)])
    KSZ = sum(bt * P * 4 * D for (_, _, _, bt) in blocks)     # int8 elems
    VSZ = sum(bt * 4 * P * DV for (_, _, _, bt) in blocks)

    in_maps = []
    for c in range(NCORES):
        kt_c = np.zeros((P, KSZ // P), dtype=np.int8)   # [d, flat]
        vp_c = np.zeros((P, VSZ // P), dtype=np.int8)   # [tok%128, flat]
        scb_c = np.zeros((P, 2, NTT, 8), dtype=np.float32)
        qt_c = np.zeros((P, SLOTS * 32), dtype=BF16)
        # gather per slot once
        kgs = {}; vgs = {}
        for s in range(SLOTS):
            b = int(assign[c, s])
            n = ns[s]
            nt = n * P
            cl = int(ctx[b])
            flat = (bt_tab[b][:, None] * BLOCK_SIZE
                    + np.arange(BLOCK_SIZE, dtype=np.int64)[None, :]).reshape(-1)[:nt]
            valid = (np.arange(nt) < cl)
            kgs[s] = kc[flat] * valid[:, None, None]   # [nt, KVH, D] int8
            vg = vc[flat] * valid[:, None, None]
            # [n, P, KVH, DV] with mask col
            vgm = np.zeros((n, P, KVH, DV), dtype=np.int8)
            vgm[:, :, :, :D] = vg.reshape(n, P, KVH, D)
            vgm[:, :, :, D] = valid.reshape(n, P)[:, :, None]
            vgs[s] = vgm
            o = int(offs[s])
            ksg = (ks[flat] * SCALE) * valid[:, None]        # [nt, KVH]
            vsg = vs[flat] * valid[:, None]
            scb_c[:, :, o: o + n, 0:4] = (
                ksg.reshape(n, P, 2, 4).transpose(1, 2, 0, 3))
            scb_c[:, :, o: o + n, 4:8] = (
                vsg.reshape(n, P, 2, 4).transpose(1, 2, 0, 3))
            qt_c[:, s * 32: (s + 1) * 32] = q[b].transpose(1, 0).astype(BF16)
        ko = vo = 0
        for (s, g, bo, bt) in blocks:
            t0, t1 = bo * P, (bo + bt) * P
            # K block: [d, j, t] contiguous per partition d
            kb = kgs[s][t0:t1, 4 * g: 4 * g + 4, :].transpose(2, 1, 0)  # [D,4,btP]
            ksz = 4 * bt * P
            kt_c[:, ko: ko + ksz] = kb.reshape(D, ksz)
            ko += ksz
            # V block: [tok%128, j, i, c] contiguous per partition
            vb = vgs[s][bo: bo + bt, :, 4 * g: 4 * g + 4, :].transpose(1, 2, 0, 3)
            vsz = 4 * bt * DV
            vp_c[:, vo: vo + vsz] = vb.reshape(P, vsz)
            vo += vsz
        in_maps.append(dict(kt=kt_c, vp=vp_c, scb=scb_c, qt=qt_c))
    return in_maps


# ---------------------------------------------------------------------------
# device program
# ---------------------------------------------------------------------------

def _build_program(ns):
    blocks = _blocks(ns)
    NTT = sum(ns)
    offs = [0]
    for n in ns:
        offs.append(offs[-1] + n)
    KSZ = sum(bt * P * 4 * D for (_, _, _, bt) in blocks)
    VSZ = sum(bt * 4 * P * DV for (_, _, _, bt) in blocks)
    f32 = mybir.dt.float32
    bf16 = mybir.dt.bfloat16
    i8 = mybir.dt.int8
    EXP = mybir.ActivationFunctionType.Exp

    nc = bacc.Bacc("TRN2", target_bir_lowering=False, debug=False,
                   num_devices=NCORES)

    kt_d = nc.dram_tensor("kt", [P, KSZ // P], i8, kind="ExternalInput").ap()
    vp_d = nc.dram_tensor("vp", [P, VSZ // P], i8, kind="ExternalInput").ap()
    scb_d = nc.dram_tensor("scb", [P, 2, NTT, 8], f32, kind="ExternalInput").ap()
    qt_d = nc.dram_tensor("qt", [P, SLOTS * 32], bf16, kind="ExternalInput").ap()
    out_d = nc.dram_tensor("out", [SLOTS, 2, P, DV], f32,
                           kind="ExternalOutput").ap()

    with tile.TileContext(nc) as tc, ExitStack() as ctx:
        const = ctx.enter_context(tc.tile_pool(name="const", bufs=1))
        kt_pool = ctx.enter_context(tc.tile_pool(name="ktp", bufs=4))
        v_pool = ctx.enter_context(tc.tile_pool(name="vpp", bufs=4))
        sc_pool = ctx.enter_context(tc.tile_pool(name="scp", bufs=2))
        work = ctx.enter_context(tc.tile_pool(name="wrk", bufs=3))
        o_pool = ctx.enter_context(tc.tile_pool(name="osb", bufs=2))
        ps_qk = ctx.enter_context(tc.tile_pool(name="psqk", bufs=3, space="PSUM"))
        ps_pv = ctx.enter_context(tc.tile_pool(name="pspv", bufs=2, space="PSUM"))

        qt = const.tile([P, SLOTS * 32], bf16)
        nc.sync.dma_start(qt, qt_d)

        ko = vo = 0
        cur = None   # (s, g) -> per-group state
        pv = scb = None
        nblk = {}
        for (s, g, bo, bt) in blocks:
            n = ns[s]
            o = offs[s]
            if cur != (s, g):
                cur = (s, g)
                scb = sc_pool.tile([P, n, 8], f32, tag="scb")
                nc.sync.dma_start(scb, scb_d[:, g, o: o + n, :])
                pv = ps_pv.tile([P, DV], f32, tag="pv")
                nc.vector.memset(pv, 0.0)
                nblk[cur] = sum(1 for (s2, g2, _, _) in blocks
                                if (s2, g2) == cur)
                blk_i = 0
            ksz = 4 * bt * P
            kc = kt_pool.tile([P, 4, bt, P], bf16, tag="kt")
            nc.gpsimd.dma_start(
                kc, kt_d[:, ko: ko + ksz].rearrange(
                    "d (j i t) -> d j i t", j=4, i=bt))
            ko += ksz
            vsz = 4 * bt * DV
            vc = v_pool.tile([P, 4, bt, DV], bf16, tag="vt")
            nc.gpsimd.dma_start(
                vc, vp_d[:, vo: vo + vsz].rearrange(
                    "p (j i c) -> p j i c", j=4, i=bt))
            vo += vsz

            qk = ps_qk.tile([P, bt, 4, 4], f32, tag="qk")
            for i in range(bt):
                for j2 in range(4):
                    qcol = s * 32 + (4 * g + j2) * 4
                    nc.tensor.matmul(
                        qk[:, i, j2, :],
                        lhsT=kc[:, j2, i, :],
                        rhs=qt[:, qcol: qcol + 4],
                        start=True, stop=True, skip_group_check=True)

            nc.vector.tensor_mul(
                qk, qk,
                scb[:, bo: bo + bt, 0:4].unsqueeze(3).to_broadcast(
                    [P, bt, 4, 4]))
            ew = work.tile([P, bt, 4, 8], bf16, tag="ew")
            nc.scalar.activation(ew[:, :, :, 0:4], qk, EXP)
            nc.vector.tensor_mul(
                ew[:, :, :, 4:8], ew[:, :, :, 0:4],
                scb[:, bo: bo + bt, 4:8].unsqueeze(3).to_broadcast(
                    [P, bt, 4, 4]))

            last = blk_i == nblk[cur] - 1
            for i in range(bt):
                for j2 in range(4):
                    nc.tensor.matmul(
                        pv[32 * j2: 32 * j2 + 8, :],
                        lhsT=ew[:, i, j2, :],
                        rhs=vc[:, j2, i, :],
                        start=(blk_i == 0 and i == 0),
                        stop=(last and i == bt - 1),
                        tile_position=(0, 32 * j2),
                        skip_group_check=True)
            blk_i += 1
            if last:
                osb = o_pool.tile([P, DV], f32, tag="osb")
                nc.vector.tensor_copy(osb, pv)
                nc.sync.dma_start(out_d[s, g], osb)

    nc.compile()
    return nc


_PROGRAM_CACHE = {}


def _get_program(ns):
    key = tuple(ns)
    if key not in _PROGRAM_CACHE:
        _PROGRAM_CACHE[key] = _build_program(ns)
    return _PROGRAM_CACHE[key]


# ---------------------------------------------------------------------------
# entry point
# ---------------------------------------------------------------------------

def kernel(q, k, v, k_cache_q, v_cache_q, k_scale, v_scale,
           block_tables, context_lens, slot_mapping, _trace=False):
    inputs = dict(q=np.asarray(q), k=np.asarray(k), v=np.asarray(v),
                  k_cache_q=np.asarray(k_cache_q),
                  v_cache_q=np.asarray(v_cache_q),
                  k_scale=np.asarray(k_scale), v_scale=np.asarray(v_scale),
                  block_tables=np.asarray(block_tables),
                  context_lens=np.asarray(context_lens),
                  slot_mapping=np.asarray(slot_mapping))
    assign, ns = _plan(inputs["context_lens"])
    blocks = _blocks(ns)
    in_maps = _pack_inputs(inputs, assign, ns, blocks)
    nc = _get_program(ns)
    res = run_bass_kernel_spmd(nc, in_maps, core_ids=list(range(NCORES)),
                               trace=_trace)

    out = np.zeros((B, NUM_HEADS, D), dtype=np.float32)
    for c in range(NCORES):
        oc = res.results[c]["out"]  # [SLOTS, 2, P, DV] f32
        for s in range(SLOTS):
            b = int(assign[c, s])
            for g in range(2):
                for j2 in range(4):
                    j = 4 * g + j2
                    z = oc[s, g, 32 * j2: 32 * j2 + 4, D]          # [4]
                    pvv = oc[s, g, 32 * j2 + 4: 32 * j2 + 8, :D]   # [4, D]
                    out[b, 4 * j: 4 * j + 4] = pvv / z[:, None]
    out = out.reshape(B, NUM_HEADS * D)
    if _trace:
        return out, res
    return out


# revision 5
# speedup vs baseline: 2.1661x; 1.0641x over previous
"""Trainium2 Bass kernel: paged int8-KV-cache GQA decode attention, 8-core SPMD.

Contract: kernel(**inputs) takes the FULL unsharded numpy inputs (as produced by
the reference setup_inputs) and returns the FULL [32, 4096] float32 output.

Strategy (data parallel + split-K over token windows):
  - Work units are (sequence, token-window) RANGES, flash-decoding style:
    every core runs an identical program over R ranges of compile-time tile
    counts rs[i]; a host-side solver cuts the 32 sequences' token streams
    into 8 windows per range size so that padding is ~the global remainder
    (<1%), vs ~12% for whole-sequence slotting.  Per-range partial (PV, Z)
    pairs are summed per sequence on the host (exact: no max-subtraction is
    used, so partial softmax sums add linearly).
  - K/V int8 cache values are gathered per block_tables into per-core packed
    int8 buffers (1 byte per element in HBM), laid out block-major so every
    HBM->SBUF DMA is one contiguous run per partition.  SWDGE DMAs cast
    int8 -> bf16 inline (exact: values are +-127 integers).
  - Work is chopped into <=BT-token-tile blocks per (range, kvh-group); a
    tiny lead block primes the pipeline so the PE starts ~2us in.
  - Per block: QK matmuls (K^T tile as stationary operand) -> one DVE mul by
    k_scale*softmax_scale -> one ACT exp -> one DVE mul by v_scale; then the
    PV+Z fused matmul lhsT=[e|ev], rhs=[V|mask] accumulated into a per-
    (range,group) PSUM bank, kvh j2 on PE column-group j2 (tile_position).
    Rows 32*j2+0..3 hold Z (col 128); rows 32*j2+4..7 hold PV (cols 0..127).
  Softmax skips max-subtraction (scores are O(20) at most; fp32 exp is safe).
"""

import math
import random
import sys
from contextlib import ExitStack

import numpy as np

sys.path.insert(0, "/opt/trn_rl_repo")

import ml_dtypes  # noqa: E402

import concourse.bass as bass  # noqa: E402
import concourse.mybir as mybir  # noqa: E402
import concourse.tile as tile  # noqa: E402
from concourse import bacc  # noqa: E402
from concourse.bass_utils import run_bass_kernel_spmd  # noqa: E402

BF16 = ml_dtypes.bfloat16

B = 32
NUM_HEADS = 32
KVH = 8
D = 128
REP = NUM_HEADS // KVH  # 4
BLOCK_SIZE = 256
T = 4096
P = 128
DV = D + 1  # V columns + mask column
SCALE = 1.0 / float(np.sqrt(D))
NCORES = 8
BT = 12     # token tiles per pipeline block


# ---------------------------------------------------------------------------
# host-side planning
# ---------------------------------------------------------------------------

def _greedy_assign(sizes, tiles):
    """Cut sequences' tile tails into 8 windows per range size (desc order).

    Returns (pad, plan) where plan[range_index] is a list of up to 8
    (seq, w0_tile, take) triples (core order), or None entries.
    """
    rem = [(int(t), b) for b, t in enumerate(tiles)]
    pad = 0
    order = np.argsort([-s for s in sizes], kind="stable")
    plan = [[None] * NCORES for _ in sizes]
    for ri in order:
        r = sizes[ri]
        for c in range(NCORES):
            rem.sort(reverse=True)
            t0, b = rem[0]
            if t0 == 0:
                pad += r
                continue
            take = min(r, t0)
            pad += r - take
            rem[0] = (t0 - take, b)
            plan[ri][c] = (b, t0 - take, take)
    left = sum(t for t, b in rem)
    return (pad if left == 0 else None), plan


def _plan(context_lens):
    """Choose shared range sizes + (core, range) -> (seq, window) assignment.

    Padding is exactly NCORES*sum(rs) - total for any feasible plan, so
    search ascending per-core totals and take the first feasible config.
    """
    tiles = [int(math.ceil(int(c) / P)) for c in context_lens]
    total = sum(tiles)
    lo = (total + NCORES - 1) // NCORES
    rng = random.Random(0)
    for tot in range(lo, lo + 13):
        for R in range(4, 11):
            if R > tot:
                break
            for _ in range(3000):
                cuts = sorted(rng.sample(range(1, tot), R - 1)) if R > 1 else []
                s = [b - a for a, b in zip([0] + cuts, cuts + [tot])]
                if max(s) > 32:
                    continue
                s.sort(reverse=True)
                pad, plan = _greedy_assign(s, tiles)
                if pad is not None:
                    return list(s), plan
    raise AssertionError("range planner failed")


def _blocks(rs):
    """[(ri, g, tile_off_in_range, bt)], with a small lead block to prime."""
    out = []
    for ri, n in enumerate(rs):
        for g in range(2):
            bo = 0
            if ri == 0 and g == 0 and n > 2:
                out.append((ri, g, 0, 2))
                bo = 2
            while bo < n:
                bt = min(BT, n - bo)
                out.append((ri, g, bo, bt))
                bo += bt
    return out


def _quantize(x):
    absmax = np.abs(x).max(axis=-1)
    scale = np.where(absmax > 0.0, absmax / 127.0, 1.0).astype(np.float32)
    xq = np.clip(np.round(x / scale[..., None]), -127.0, 127.0).astype(np.int8)
    return xq, scale


def _pack_inputs(inputs, rs, plan, blocks):
    q = inputs["q"].reshape(B, NUM_HEADS, D).astype(np.float32)
    k = inputs["k"].reshape(B, KVH, D).astype(np.float32)
    v = inputs["v"].reshape(B, KVH, D).astype(np.float32)
    kc = np.ascontiguousarray(
        inputs["k_cache_q"].reshape(-1, KVH, D).astype(np.int8))
    vc = np.ascontiguousarray(
        inputs["v_cache_q"].reshape(-1, KVH, D).astype(np.int8))
    ks = np.ascontiguousarray(inputs["k_scale"].reshape(-1, KVH)).astype(np.float32)
    vs = np.ascontiguousarray(inputs["v_scale"].reshape(-1, KVH)).astype(np.float32)
    bt_tab = inputs["block_tables"]
    ctx = inputs["context_lens"]
    sm = inputs["slot_mapping"]

    # store_kvcache_int8: quantize the new token and scatter into the cache
    kq, ksn = _quantize(k)
    vq, vsn = _quantize(v)
    kc = kc.copy(); vc = vc.copy(); ks = ks.copy(); vs = vs.copy()
    kc[sm] = kq; vc[sm] = vq; ks[sm] = ksn; vs[sm] = vsn

    R = len(rs)
    RT = sum(rs)
    offs = np.concatenate([[0], np.cumsum(rs)])
    KSZ = sum(bt * P * 4 * D for (_, _, _, bt) in blocks)     # int8 elems
    VSZ = sum(bt * 4 * P * DV for (_, _, _, bt) in blocks)

    # gather + zero-pad each sequence once, globally
    kg_all = {}; vg_all = {}; ksg_all = {}; vsg_all = {}
    for b in range(B):
        nt = int(math.ceil(int(ctx[b]) / P)) * P
        cl = int(ctx[b])
        flat = (bt_tab[b][:, None] * BLOCK_SIZE
                + np.arange(BLOCK_SIZE, dtype=np.int64)[None, :]).reshape(-1)[:nt]
        valid = (np.arange(nt) < cl)
        kg_all[b] = kc[flat] * valid[:, None, None]          # [nt, KVH, D]
        vg = vc[flat] * valid[:, None, None]
        n = nt // P
        vgm = np.zeros((n, P, KVH, DV), dtype=np.int8)
        vgm[:, :, :, :D] = vg.reshape(n, P, KVH, D)
        vgm[:, :, :, D] = valid.reshape(n, P)[:, :, None]
        vg_all[b] = vgm
        ksg_all[b] = (ks[flat] * SCALE) * valid[:, None]     # [nt, KVH]
        vsg_all[b] = vs[flat] * valid[:, None]

    in_maps = []
    for c in range(NCORES):
        kt_c = np.zeros((P, KSZ // P), dtype=np.int8)   # [d, flat]
        vp_c = np.zeros((P, VSZ // P), dtype=np.int8)   # [tok%128, flat]
        scb_c = np.zeros((P, 2, RT, 8), dtype=np.float32)
        qt_c = np.zeros((P, R * 32), dtype=BF16)
        # stage per-range gathered windows (padded to rs[ri] tiles)
        kw = {}; vw = {}
        for ri in range(R):
            n = rs[ri]
            o = int(offs[ri])
            w = plan[ri][c]
            kwin = np.zeros((n * P, KVH, D), dtype=np.int8)
            vwin = np.zeros((n, P, KVH, DV), dtype=np.int8)
            if w is not None:
                b, w0, take = w
                kwin[: take * P] = kg_all[b][w0 * P: (w0 + take) * P]
                vwin[: take] = vg_all[b][w0: w0 + take]
                ksgw = np.zeros((n * P, KVH), dtype=np.float32)
                vsgw = np.zeros((n * P, KVH), dtype=np.float32)
                ksgw[: take * P] = ksg_all[b][w0 * P: (w0 + take) * P]
                vsgw[: take * P] = vsg_all[b][w0 * P: (w0 + take) * P]
                scb_c[:, :, o: o + n, 0:4] = (
                    ksgw.reshape(n, P, 2, 4).transpose(1, 2, 0, 3))
                scb_c[:, :, o: o + n, 4:8] = (
                    vsgw.reshape(n, P, 2, 4).transpose(1, 2, 0, 3))
                qt_c[:, ri * 32: (ri + 1) * 32] = q[b].transpose(1, 0).astype(BF16)
            kw[ri] = kwin
            vw[ri] = vwin
        ko = vo = 0
        for (ri, g, bo, bt) in blocks:
            t0, t1 = bo * P, (bo + bt) * P
            kb = kw[ri][t0:t1, 4 * g: 4 * g + 4, :].transpose(2, 1, 0)  # [D,4,btP]
            ksz = 4 * bt * P
            kt_c[:, ko: ko + ksz] = kb.reshape(D, ksz)
            ko += ksz
            vb = vw[ri][bo: bo + bt, :, 4 * g: 4 * g + 4, :].transpose(1, 2, 0, 3)
            vsz = 4 * bt * DV
            vp_c[:, vo: vo + vsz] = vb.reshape(P, vsz)
            vo += vsz
        in_maps.append(dict(kt=kt_c, vp=vp_c, scb=scb_c, qt=qt_c))
    return in_maps


# ---------------------------------------------------------------------------
# device program
# ---------------------------------------------------------------------------

def _build_program(rs):
    blocks = _blocks(rs)
    R = len(rs)
    RT = sum(rs)
    offs = [0]
    for n in rs:
        offs.append(offs[-1] + n)
    KSZ = sum(bt * P * 4 * D for (_, _, _, bt) in blocks)
    VSZ = sum(bt * 4 * P * DV for (_, _, _, bt) in blocks)
    f32 = mybir.dt.float32
    bf16 = mybir.dt.bfloat16
    i8 = mybir.dt.int8
    EXP = mybir.ActivationFunctionType.Exp

    nc = bacc.Bacc("TRN2", target_bir_lowering=False, debug=False,
                   num_devices=NCORES)

    kt_d = nc.dram_tensor("kt", [P, KSZ // P], i8, kind="ExternalInput").ap()
    vp_d = nc.dram_tensor("vp", [P, VSZ // P], i8, kind="ExternalInput").ap()
    scb_d = nc.dram_tensor("scb", [P, 2, RT, 8], f32, kind="ExternalInput").ap()
    qt_d = nc.dram_tensor("qt", [P, R * 32], bf16, kind="ExternalInput").ap()
    out_d = nc.dram_tensor("out", [R, 2, P, DV], f32,
                           kind="ExternalOutput").ap()

    with tile.TileContext(nc) as tc, ExitStack() as ctx:
        const = ctx.enter_context(tc.tile_pool(name="const", bufs=1))
        kt_pool = ctx.enter_context(tc.tile_pool(name="ktp", bufs=4))
        v_pool = ctx.enter_context(tc.tile_pool(name="vpp", bufs=4))
        sc_pool = ctx.enter_context(tc.tile_pool(name="scp", bufs=2))
        work = ctx.enter_context(tc.tile_pool(name="wrk", bufs=3))
        o_pool = ctx.enter_context(tc.tile_pool(name="osb", bufs=2))
        ps_qk = ctx.enter_context(tc.tile_pool(name="psqk", bufs=3, space="PSUM"))
        ps_pv = ctx.enter_context(tc.tile_pool(name="pspv", bufs=2, space="PSUM"))

        qt = const.tile([P, R * 32], bf16)
        nc.sync.dma_start(qt, qt_d)

        ko = vo = 0
        cur = None
        pv = scb = None
        blk_i = 0
        nblk = {}
        for (s2, g2, _, _) in blocks:
            nblk[(s2, g2)] = nblk.get((s2, g2), 0) + 1
        for (ri, g, bo, bt) in blocks:
            n = rs[ri]
            o = offs[ri]
            if cur != (ri, g):
                cur = (ri, g)
                scb = sc_pool.tile([P, n, 8], f32, tag="scb")
                nc.sync.dma_start(scb, scb_d[:, g, o: o + n, :])
                pv = ps_pv.tile([P, DV], f32, tag="pv")
                nc.vector.memset(pv, 0.0)
                blk_i = 0
            ksz = 4 * bt * P
            kc = kt_pool.tile([P, 4, bt, P], bf16, tag="kt")
            nc.gpsimd.dma_start(
                kc, kt_d[:, ko: ko + ksz].rearrange(
                    "d (j i t) -> d j i t", j=4, i=bt))
            ko += ksz
            vsz = 4 * bt * DV
            vc = v_pool.tile([P, 4, bt, DV], bf16, tag="vt")
            nc.gpsimd.dma_start(
                vc, vp_d[:, vo: vo + vsz].rearrange(
                    "p (j i c) -> p j i c", j=4, i=bt))
            vo += vsz

            qk = ps_qk.tile([P, bt, 4, 4], f32, tag="qk")
            for i in range(bt):
                for j2 in range(4):
                    qcol = ri * 32 + (4 * g + j2) * 4
                    nc.tensor.matmul(
                        qk[:, i, j2, :],
                        lhsT=kc[:, j2, i, :],
                        rhs=qt[:, qcol: qcol + 4],
                        start=True, stop=True, skip_group_check=True)

            nc.vector.tensor_mul(
                qk, qk,
                scb[:, bo: bo + bt, 0:4].unsqueeze(3).to_broadcast(
                    [P, bt, 4, 4]))
            ew = work.tile([P, bt, 4, 8], bf16, tag="ew")
            nc.scalar.activation(ew[:, :, :, 0:4], qk, EXP)
            nc.vector.tensor_mul(
                ew[:, :, :, 4:8], ew[:, :, :, 0:4],
                scb[:, bo: bo + bt, 4:8].unsqueeze(3).to_broadcast(
                    [P, bt, 4, 4]))

            last = blk_i == nblk[cur] - 1
            for i in range(bt):
                for j2 in range(4):
                    nc.tensor.matmul(
                        pv[32 * j2: 32 * j2 + 8, :],
                        lhsT=ew[:, i, j2, :],
                        rhs=vc[:, j2, i, :],
                        start=(blk_i == 0 and i == 0),
                        stop=(last and i == bt - 1),
                        tile_position=(0, 32 * j2),
                        skip_group_check=True)
            blk_i += 1
            if last:
                osb = o_pool.tile([P, DV], f32, tag="osb")
                nc.vector.tensor_copy(osb, pv)
                nc.sync.dma_start(out_d[ri, g], osb)

    nc.compile()
    return nc


_PROGRAM_CACHE = {}
_PLAN_CACHE = {}


def _get_program(rs):
    key = tuple(rs)
    if key not in _PROGRAM_CACHE:
        _PROGRAM_CACHE[key] = _build_program(rs)
    return _PROGRAM_CACHE[key]


# ---------------------------------------------------------------------------
# entry point
# ---------------------------------------------------------------------------

def kernel(q, k, v, k_cache_q, v_cache_q, k_scale, v_scale,
           block_tables, context_lens, slot_mapping, _trace=False):
    inputs = dict(q=np.asarray(q), k=np.asarray(k), v=np.asarray(v),
                  k_cache_q=np.asarray(k_cache_q),
                  v_cache_q=np.asarray(v_cache_q),
                  k_scale=np.asarray(k_scale), v_scale=np.asarray(v_scale),
                  block_tables=np.asarray(block_tables),
                  context_lens=np.asarray(context_lens),
                  slot_mapping=np.asarray(slot_mapping))
    ctx_key = inputs["context_lens"].tobytes()
    if ctx_key not in _PLAN_CACHE:
        _PLAN_CACHE[ctx_key] = _plan(inputs["context_lens"])
    rs, plan = _PLAN_CACHE[ctx_key]
    blocks = _blocks(rs)
    in_maps = _pack_inputs(inputs, rs, plan, blocks)
    nc = _get_program(rs)
    res = run_bass_kernel_spmd(nc, in_maps, core_ids=list(range(NCORES)),
                               trace=_trace)

    num = np.zeros((B, NUM_HEADS, D), dtype=np.float64)
    den = np.zeros((B, NUM_HEADS), dtype=np.float64)
    R = len(rs)
    for c in range(NCORES):
        oc = res.results[c]["out"]  # [R, 2, P, DV] f32
        for ri in range(R):
            w = plan[ri][c]
            if w is None:
                continue
            b = w[0]
            for g in range(2):
                for j2 in range(4):
                    j = 4 * g + j2
                    den[b, 4 * j: 4 * j + 4] += oc[ri, g, 32 * j2: 32 * j2 + 4, D]
                    num[b, 4 * j: 4 * j + 4] += oc[ri, g,
                                                   32 * j2 + 4: 32 * j2 + 8, :D]
    out = (num / den[:, :, None]).astype(np.float32).reshape(B, NUM_HEADS * D)
    if _trace:
        return out, res
    return out
